# revision 30
# baseline (speedup 1.0000x reference)
"""AdaptiveWingLoss on 8 TRN2 NeuronCores (Bass/Tile) — "K1" tangent-correction
formulation.

Shards batch (8) across cores; each core computes the weighted loss sum over
its 68 maps of 128x128; host combines partial sums into the mean.

Math (ALPHA=2.1, OMEGA=14, THETA=0.5, EPS=1, W=10), amy = 2.1 - t:
  dY   = max(|p - t|, 0.004);  dm = 2*dY
  lnd  = ln(dY) = Ln(0.5*dm)
  q    = amy*lnd  (<= 0);  eq = exp(q) = dY**amy;  sS = ln(1+eq) = l14_small
  The large branch (dY >= 0.5) is the tangent line of sS at dY=0.5, so
  l14 = sS + corr with corr = relu(dm-1)^2 * (c0 + c1*t) — a least-squares fit
  of the tangency gap (|fit err| < 9e-3, final mean rel err ~2e-5).
  No branch select needed: relu(dm-1)^2 vanishes on the small branch.
  wfb  = 3x3 binary dilation of b=[t>=0.2] (borders keep b), w = wfb + 0.1
  loss = 140 * sum(w * l14) / N

Engines: ACT does {Copy(amy), Ln, Exp, Ln} — one table set, one
ACT_TABLE_LOAD, no phase gates. DVE does 3 full custom 1x passes (ABSDM, CORR,
WACC drain) plus stock bf16 ops (b threshold 4x; q, l14 at 2x). GpSimd (Pool)
does only the b_g pad memsets — its elementwise rate is ~15x too slow for
full-res work and its SBUF traffic degrades concurrent DVE perf modes. PE does
the 3x3 OR-dilation as 3 shifted band-matrix matmuls per 512-col window
(center matrix carries the border-row identity). The WACC drain fuses
threshold+weight+multiply+reduce: ((ps>=0.5)+0.1)*l14 with accum=ADD over
768-col PSUM windows, so the weighted sum falls out of the drain directly;
border columns (which must keep b instead of the dilated value) are patched by
two tiny ::127-strided passes per window (subtract the wrong term, add the
right one). DMA: targets on sync HWDGE, predictions on scalar HWDGE — both
queues' sequencers block for the whole transfer, so loads never share a queue
with the hot compute engines (DVE), and gpsimd SWDGE measured strictly slower
for these [h, m, w]-strided loads.

Measured (core 0): ~70.5us exec; DVE busy ~46us (the critical engine), ACT
~36.5us, PE ~26us, DMA ~30us/engine. Baseline before rework: 90.9us.
"""

import numpy as np

import concourse.bass as bass
import concourse.tile as tile
from concourse import bacc
from concourse import mybir

F32 = mybir.dt.float32
BF16 = mybir.dt.bfloat16
AF = mybir.ActivationFunctionType
ALU = mybir.AluOpType

H = 128
N_CORES = 8
N_MAPS = 68  # per core (68 landmarks x 1 batch element)
# graded so DMA never starves compute: per queue a chunk loads in ~0.34us/map
# while DVE consumes ~0.68us/map, so each chunk's load must fit under the
# accumulated compute cover: 0.34*s_i <= 1 + 0.336*sum(s_j, j<i)
SIZES = (2, 4, 8, 12, 12, 12, 12, 6)

# corr = relu(dm-1)^2 * (CC0 + CC1*t), least-squares fit of the tangency gap
# over the (t, dY) distribution. Reparametrized in amy = 2.1 - t at the call
# site: s0 = CC0 + 2.1*CC1, s1 = -CC1.
CC0 = -0.097028
CC1 = 0.140812

_ALLOWED_TABLES = ("natural_log_exp_and_others",)
_patched_tables = False
_custom_ops = {}


def _register_custom_ops():
    """Four fused DVE ops:
    AWL_ABSDM:  out = max(max(in0-in1, in1-in0), s0) * imm2     (dm = 2*dY)
    AWL_CORR:   out = relu(in0-1)^2 * (s1*in1 + s0)             (tangency fix)
    AWL_WACC:   out = ((in0>=s0)+s1)*in1, accum_out = sum(out)  (weighted drain)
    AWL_WACCN:  out = -((in0>=s0)+s1)*in1, accum_out = sum(out) (border undo)
    """
    if _custom_ops:
        return _custom_ops
    from concourse import dve_ops
    from concourse.dve_spec import (
        Spec, Src0, Src1, C0, C1, C2, Zero, One, maxx, relu, lower, AluOp,
    )
    from concourse.dve_uop import DveOpSpec

    r = relu(Src0 - One)
    defs = [
        (
            "AWL_ABSDM",
            Spec(
                body=maxx(maxx(Src0 - Src1, Src1 - Src0), C0) * C2,
                reference=lambda in0, in1, s0, s1, imm2: (
                    np.maximum(np.abs(in0.astype(np.float32) - in1), s0) * imm2
                ).astype(np.float32),
            ),
        ),
        (
            "AWL_CORR",
            Spec(
                body=(r * (C1 * Src1 + C0)) * r,
                reference=lambda in0, in1, s0, s1, imm2: (
                    (np.maximum(in0.astype(np.float32) - 1.0, 0.0) ** 2)
                    * (s1 * in1 + s0)
                ).astype(np.float32),
            ),
        ),
        (
            "AWL_WACC",
            Spec(
                body=((Src0 >= C0) + C1) * Src1,
                accum=AluOp.ADD,
                reference=lambda in0, in1, s0, s1, imm2: (
                    ((in0 >= s0).astype(np.float32) + s1) * in1
                ).astype(np.float32),
            ),
        ),
        (
            "AWL_WACCN",
            Spec(
                body=(Zero - ((Src0 >= C0) + C1)) * Src1,
                accum=AluOp.ADD,
                reference=lambda in0, in1, s0, s1, imm2: (
                    -((in0 >= s0).astype(np.float32) + s1) * in1
                ).astype(np.float32),
            ),
        ),
    ]
    for name, spec in defs:
        if name in dve_ops._SUB_OPCODE_FOR_NAME:
            _custom_ops[name] = next(o for o in dve_ops.OPS if o.name == name)
            continue
        opcode = dve_ops._CUSTOM_DVE_ROW_BASE + len(dve_ops.OPS)
        assert opcode < 0x20
        shas = {}
        for ver in ("v3", "v4"):
            ds = DveOpSpec(
                name=name, opcode=opcode, uops=lower(spec, ver=ver), rd1_en=True
            )
            shas[ver] = ds.sha(ver)
        dve_ops._SUB_OPCODE_FOR_NAME[name] = opcode
        op = dve_ops.DveOp(name, spec, subdim=False, uops_sha=shas)
        dve_ops.OPS.append(op)
        dve_ops.CUSTOM_DVE_SPECS[name] = spec
        _custom_ops[name] = op
    return _custom_ops


def _patch_act_tables():
    """Restrict bacc's activation-set choices to the single set we use
    (Ln+Exp) so the fixpoint pass emits exactly one ACT_TABLE_LOAD."""
    global _patched_tables
    if _patched_tables:
        return
    orig = bacc.get_activation_tables

    def patched(arch):
        tabs = orig(arch)
        return {k: (v if k in _ALLOWED_TABLES else set()) for k, v in tabs.items()}

    bacc.get_activation_tables = patched
    _patched_tables = True


def make_vband():
    """[128, 256] stationaries for the vertical OR:
    cols 0:128   M_side   = 3-row band, interior output rows 1..126 only
    cols 128:256 M_center = band + identity at rows 0/127 (border rows keep b)."""
    v = np.zeros((H, 2 * H), dtype=np.float32)
    for i in range(1, H - 1):
        for k in (i - 1, i, i + 1):
            v[k, i] = 1.0
            v[k, H + i] = 1.0
    v[0, H + 0] = 1.0
    v[H - 1, H + H - 1] = 1.0
    return v


def build_nc(n_maps=N_MAPS, sizes=SIZES):
    _patch_act_tables()
    ops = _register_custom_ops()
    assert sum(sizes) == n_maps
    chunks = []
    m0 = 0
    for c in sizes:
        chunks.append((m0, c))
        m0 += c
    nch = len(chunks)
    cm = max(sizes)

    # accumulator columns: 3 per 768-col drain window
    n_acc = 3 * sum(-(-c * H // 768) for c in sizes)

    nc = bacc.Bacc("TRN2")
    pred = nc.declare_dram_parameter("predictions", [n_maps, H, H], F32, isOutput=False)
    targ = nc.declare_dram_parameter("targets", [n_maps, H, H], F32, isOutput=False)
    vband = nc.declare_dram_parameter("vband", [H, 2 * H], BF16, isOutput=False)
    outd = nc.declare_dram_parameter("out", [H, n_acc], F32, isOutput=True)

    with tile.TileContext(nc) as tc:
        with (
            tc.tile_pool(name="io", bufs=2) as iop,
            tc.tile_pool(name="wk", bufs=2) as wk,
            tc.tile_pool(name="acc", bufs=1) as accp,
            tc.tile_pool(name="psum", bufs=4, space="PSUM") as psp,
        ):
            acc = accp.tile([H, n_acc], F32, tag="acc", name="acc")
            # band stationaries (bf16, converted host-side)
            vb = accp.tile([H, 2 * H], BF16, tag="vb", name="vb")
            nc.sync.dma_start(out=vb[:], in_=vband[:])
            b_gs = []
            for k in range(2):
                bg = accp.tile([H, cm * H + 4], BF16, tag=f"b_g{k}", name=f"b_g{k}")[:]
                nc.gpsimd.memset(bg, 0.0)
                b_gs.append(bg)

            tiles = {}

            def phase1(ci, m0, c):
                F = c * H
                # sync + scalar HWDGE queues; a queue's sequencer blocks for
                # the whole transfer (~4us/786KB), and gpsimd SWDGE measured
                # strictly slower for these strided loads.
                tt = iop.tile([H, F], F32, tag="tt", name=f"tt{ci}")
                nc.sync.dma_start(
                    out=tt[:].rearrange("p (m w) -> p m w", w=H),
                    in_=targ[m0 : m0 + c].rearrange("m h w -> h m w"),
                )
                tp = iop.tile([H, F], F32, tag="tp", name=f"tp{ci}")
                nc.scalar.dma_start(
                    out=tp[:].rearrange("p (m w) -> p m w", w=H),
                    in_=pred[m0 : m0 + c].rearrange("m h w -> h m w"),
                )
                tt, tp = tt[:], tp[:]
                b_g = b_gs[ci % 2]
                amy = wk.tile([H, F], BF16, tag="amy", name=f"amy{ci}")[:]
                # amy = 2.1 - tt (ACT Copy with free affine; Pool is ~15x too
                # slow for full-res elementwise and wrecks DVE SBUF ports)
                nc.scalar.activation(amy, tt, AF.Copy, bias=2.1, scale=-1.0)
                # dm first in DVE order: b would chain behind amy (ACT) and
                # stall the in-order DVE queue at startup
                dm = wk.tile([H, F], BF16, tag="dm", name=f"dm{ci}")[:]
                nc.vector._custom_dve(
                    ops["AWL_ABSDM"], out=dm, in0=tp, in1=tt, s0=0.004, imm2=2.0
                )
                # b = [t >= 0.2] = [amy <= 1.9] into the padded b_g (bf16 4x)
                nc.vector.tensor_scalar(
                    b_g[:, 2 : F + 2], amy, 1.9, None, ALU.is_le
                )
                tiles[ci] = (dm, amy)

            def phase2(ci, m0, c):
                F = c * H
                dm, amy = tiles[ci]

                def T(tag):
                    return wk.tile([H, F], BF16, tag=tag, name=f"{tag}{ci}")[:]

                lnd, q, eq, sS, corr, l14 = (
                    T("lnd"), T("q"), T("eq"), T("sS"), T("corr"), T("l14"),
                )
                nc.scalar.activation(lnd, dm, AF.Ln, scale=0.5)
                nc.vector.tensor_tensor(q, amy, lnd, ALU.mult)
                nc.scalar.activation(eq, q, AF.Exp)
                nc.scalar.activation(sS, eq, AF.Ln, bias=1.0)
                nc.vector._custom_dve(
                    ops["AWL_CORR"], out=corr, in0=dm, in1=amy,
                    s0=CC0 + 2.1 * CC1, s1=-CC1,
                )
                nc.vector.tensor_tensor(l14, sS, corr, ALU.add)
                tiles[ci] = l14

            kctr = [0]

            def phase3(ci, m0, c):
                F = c * H
                l14 = tiles.pop(ci)
                b_g = b_gs[ci % 2]
                waste = wk.tile([H, F], BF16, tag="waste", name=f"waste{ci}")[:]
                # 768-col drain windows: 2-bank PSUM tiles, 4 in flight; fewer
                # drain dispatches than per-512 while keeping PE/DVE pipelined
                for w0 in range(0, F, 768):
                    ww = min(768, F - w0)
                    ps = psp.tile([H, ww], F32, tag="ps", name=f"ps{ci}_{w0}")
                    for s0 in range(0, ww, 512):
                        cw = min(512, ww - s0)
                        pw = ps[:, s0 : s0 + cw]
                        c0 = w0 + s0
                        nc.tensor.matmul(
                            pw, vb[:, 0:H], b_g[:, c0 + 1 : c0 + 1 + cw],
                            start=True, stop=False,
                        )
                        nc.tensor.matmul(
                            pw, vb[:, H : 2 * H], b_g[:, c0 + 2 : c0 + 2 + cw],
                            start=False, stop=False,
                        )
                        nc.tensor.matmul(
                            pw, vb[:, 0:H], b_g[:, c0 + 3 : c0 + 3 + cw],
                            start=False, stop=True,
                        )
                    k = kctr[0]
                    kctr[0] += 3
                    lsl = l14[:, w0 : w0 + ww]
                    wsl = waste[:, w0 : w0 + ww]
                    # main drain: sum ((ps>=0.5)+0.1)*l14 over the window
                    nc.vector._custom_dve(
                        ops["AWL_WACC"], out=wsl, in0=ps[:], in1=lsl,
                        s0=0.5, s1=0.1, accum_out=acc[:, k : k + 1],
                    )
                    # border cols keep b: undo the ps-thresholded term, add
                    # the b-thresholded one (cols {0,127} of each map via
                    # ::127).  These hide in pipeline gaps — dropping them
                    # saved nothing and cost 5x accuracy (measured).
                    ps3 = ps[:].rearrange("p (m w) -> p m w", w=H)[:, :, ::H - 1]
                    l3 = lsl.rearrange("p (m w) -> p m w", w=H)[:, :, ::H - 1]
                    w3 = wsl.rearrange("p (m w) -> p m w", w=H)[:, :, ::H - 1]
                    b3 = b_g[:, w0 + 2 : w0 + 2 + ww].rearrange(
                        "p (m w) -> p m w", w=H
                    )[:, :, ::H - 1]
                    nc.vector._custom_dve(
                        ops["AWL_WACCN"], out=w3, in0=ps3, in1=l3,
                        s0=0.5, s1=0.1, accum_out=acc[:, k + 1 : k + 2],
                    )
                    nc.vector._custom_dve(
                        ops["AWL_WACC"], out=w3, in0=b3, in1=l3,
                        s0=0.5, s1=0.1, accum_out=acc[:, k + 2 : k + 3],
                    )

            # 3-deep software pipeline
            ksplit = [0]
            for i in range(nch + 2):
                if i < nch:
                    phase1(i, *chunks[i])
                if 1 <= i <= nch:
                    phase2(i - 1, *chunks[i - 1])
                if i >= 2:
                    phase3(i - 2, *chunks[i - 2])
                if i == nch:
                    # store the finished first half of acc early so only a
                    # small strip remains after the last drain
                    ksplit[0] = kctr[0]
                    nc.sync.dma_start(
                        out=outd[:, : ksplit[0]], in_=acc[:, : ksplit[0]]
                    )
            nc.sync.dma_start(
                out=outd[:, ksplit[0] :], in_=acc[:, ksplit[0] :]
            )
    nc.compile()
    return nc


_TRACE = {"enabled": False, "last": None}


def kernel(predictions, targets):
    from concourse.bass_utils import run_bass_kernel_spmd

    preds = np.ascontiguousarray(predictions, dtype=np.float32)
    targs = np.ascontiguousarray(targets, dtype=np.float32)
    B = preds.shape[0]
    import ml_dtypes
    vband = make_vband().astype(ml_dtypes.bfloat16)
    in_maps = [
        {"predictions": preds[i], "targets": targs[i], "vband": vband}
        for i in range(N_CORES)
    ]
    nc = build_nc()
    kwargs = {}
    if _TRACE["enabled"]:
        kwargs = {"trace": True}
    try:
        res = run_bass_kernel_spmd(nc, in_maps, core_ids=list(range(N_CORES)), **kwargs)
    except Exception:
        if not kwargs:
            raise
        res = run_bass_kernel_spmd(nc, in_maps, core_ids=list(range(N_CORES)))
    _TRACE["last"] = res
    tot = 0.0
    for r in res.results:
        o = np.asarray(r["out"], dtype=np.float64)
        tot += 140.0 * o.sum()
    n_total = B * N_MAPS * H * H
    return np.float32(tot / n_total)


# revision 31
# speedup vs baseline: 1.0744x; 1.0744x over previous
"""AdaptiveWingLoss on 8 TRN2 NeuronCores (Bass/Tile) — "K1" tangent-correction
formulation.

Shards batch (8) across cores; each core computes the weighted loss sum over
its 68 maps of 128x128; host combines partial sums into the mean.

Math (ALPHA=2.1, OMEGA=14, THETA=0.5, EPS=1, W=10), amy = 2.1 - t:
  dY   = max(|p - t|, 0.004);  dm = 2*dY
  lnd  = ln(dY) = Ln(0.5*dm)
  q    = amy*lnd  (<= 0);  eq = exp(q) = dY**amy;  sS = ln(1+eq) = l14_small
  The large branch (dY >= 0.5) is the tangent line of sS at dY=0.5, so
  l14 = sS + corr with corr = relu(dm-1)^2 * (c0 + c1*t) — a least-squares fit
  of the tangency gap (|fit err| < 9e-3, final mean rel err ~2e-5).
  No branch select needed: relu(dm-1)^2 vanishes on the small branch.
  wfb  = 3x3 binary dilation of b=[t>=0.2] (borders keep b), w = wfb + 0.1
  loss = 140 * sum(w * l14) / N

Engines: ACT does {Copy(amy), Ln, Exp, Ln} — one table set, one
ACT_TABLE_LOAD, no phase gates. DVE does 3 full custom 1x passes (ABSDM, CORR,
WACC drain) plus stock bf16 ops (b threshold 4x; q, l14 at 2x). GpSimd (Pool)
does only the b_g pad memsets — its elementwise rate is ~15x too slow for
full-res work and its SBUF traffic degrades concurrent DVE perf modes. PE does
the 3x3 OR-dilation as 3 shifted band-matrix matmuls per 512-col window
(center matrix carries the border-row identity). The WACC drain fuses
threshold+weight+multiply+reduce: ((ps>=0.5)+0.1)*l14 with accum=ADD over
768-col PSUM windows, so the weighted sum falls out of the drain directly;
border columns (which must keep b instead of the dilated value) are patched by
two tiny ::127-strided passes per window (subtract the wrong term, add the
right one). DMA: targets on sync HWDGE, predictions on scalar HWDGE — both
queues' sequencers block for the whole transfer, so loads never share a queue
with the hot compute engines (DVE), and gpsimd SWDGE measured strictly slower
for these [h, m, w]-strided loads.

Measured (core 0): ~70.5us exec; DVE busy ~46us (the critical engine), ACT
~36.5us, PE ~26us, DMA ~30us/engine. Baseline before rework: 90.9us.
"""

import numpy as np

import concourse.bass as bass
import concourse.tile as tile
from concourse import bacc
from concourse import mybir

F32 = mybir.dt.float32
BF16 = mybir.dt.bfloat16
AF = mybir.ActivationFunctionType
ALU = mybir.AluOpType

H = 128
N_CORES = 8
N_MAPS = 68  # per core (68 landmarks x 1 batch element)
# (2,12,...): best measured schedule.  Graded ramps (6,8,12,... / 2,4,8,...)
# trade startup vs early-gap time and measured equal or worse; more chunks
# add per-op dispatch overhead.
SIZES = (2, 12, 12, 12, 12, 12, 6)

# corr = relu(dm-1)^2 * (CC0 + CC1*t), least-squares fit of the tangency gap
# over the (t, dY) distribution. Reparametrized in amy = 2.1 - t at the call
# site: s0 = CC0 + 2.1*CC1, s1 = -CC1.
CC0 = -0.097028
CC1 = 0.140812

_ALLOWED_TABLES = ("natural_log_exp_and_others",)
_patched_tables = False
_custom_ops = {}


def _register_custom_ops():
    """Four fused DVE ops:
    AWL_ABSDM:  out = max(max(in0-in1, in1-in0), s0) * imm2     (dm = 2*dY)
    AWL_CORR:   out = relu(in0-1)^2 * (s1*in1 + s0)             (tangency fix)
    AWL_WACC:   out = ((in0>=s0)+s1)*in1, accum_out = sum(out)  (weighted drain)
    AWL_WACCN:  out = -((in0>=s0)+s1)*in1, accum_out = sum(out) (border undo)
    """
    if _custom_ops:
        return _custom_ops
    from concourse import dve_ops
    from concourse.dve_spec import (
        Spec, Src0, Src1, C0, C1, C2, Zero, One, maxx, relu, lower, AluOp,
    )
    from concourse.dve_uop import DveOpSpec

    r = relu(Src0 - One)
    defs = [
        (
            "AWL_ABSDM",
            Spec(
                body=maxx(maxx(Src0 - Src1, Src1 - Src0), C0) * C2,
                reference=lambda in0, in1, s0, s1, imm2: (
                    np.maximum(np.abs(in0.astype(np.float32) - in1), s0) * imm2
                ).astype(np.float32),
            ),
        ),
        (
            "AWL_CORR",
            Spec(
                body=(r * (C1 * Src1 + C0)) * r,
                reference=lambda in0, in1, s0, s1, imm2: (
                    (np.maximum(in0.astype(np.float32) - 1.0, 0.0) ** 2)
                    * (s1 * in1 + s0)
                ).astype(np.float32),
            ),
        ),
        (
            "AWL_WACC",
            Spec(
                body=((Src0 >= C0) + C1) * Src1,
                accum=AluOp.ADD,
                reference=lambda in0, in1, s0, s1, imm2: (
                    ((in0 >= s0).astype(np.float32) + s1) * in1
                ).astype(np.float32),
            ),
        ),
        (
            "AWL_WACCN",
            Spec(
                body=(Zero - ((Src0 >= C0) + C1)) * Src1,
                accum=AluOp.ADD,
                reference=lambda in0, in1, s0, s1, imm2: (
                    -((in0 >= s0).astype(np.float32) + s1) * in1
                ).astype(np.float32),
            ),
        ),
    ]
    for name, spec in defs:
        if name in dve_ops._SUB_OPCODE_FOR_NAME:
            _custom_ops[name] = next(o for o in dve_ops.OPS if o.name == name)
            continue
        opcode = dve_ops._CUSTOM_DVE_ROW_BASE + len(dve_ops.OPS)
        assert opcode < 0x20
        shas = {}
        for ver in ("v3", "v4"):
            ds = DveOpSpec(
                name=name, opcode=opcode, uops=lower(spec, ver=ver), rd1_en=True
            )
            shas[ver] = ds.sha(ver)
        dve_ops._SUB_OPCODE_FOR_NAME[name] = opcode
        op = dve_ops.DveOp(name, spec, subdim=False, uops_sha=shas)
        dve_ops.OPS.append(op)
        dve_ops.CUSTOM_DVE_SPECS[name] = spec
        _custom_ops[name] = op
    return _custom_ops


def _patch_act_tables():
    """Restrict bacc's activation-set choices to the single set we use
    (Ln+Exp) so the fixpoint pass emits exactly one ACT_TABLE_LOAD."""
    global _patched_tables
    if _patched_tables:
        return
    orig = bacc.get_activation_tables

    def patched(arch):
        tabs = orig(arch)
        return {k: (v if k in _ALLOWED_TABLES else set()) for k, v in tabs.items()}

    bacc.get_activation_tables = patched
    _patched_tables = True


def make_vband():
    """[128, 256] stationaries for the vertical OR:
    cols 0:128   M_side   = 3-row band, interior output rows 1..126 only
    cols 128:256 M_center = band + identity at rows 0/127 (border rows keep b)."""
    v = np.zeros((H, 2 * H), dtype=np.float32)
    for i in range(1, H - 1):
        for k in (i - 1, i, i + 1):
            v[k, i] = 1.0
            v[k, H + i] = 1.0
    v[0, H + 0] = 1.0
    v[H - 1, H + H - 1] = 1.0
    return v


def build_nc(n_maps=N_MAPS, sizes=SIZES):
    _patch_act_tables()
    ops = _register_custom_ops()
    assert sum(sizes) == n_maps
    chunks = []
    m0 = 0
    for c in sizes:
        chunks.append((m0, c))
        m0 += c
    nch = len(chunks)
    cm = max(sizes)

    # accumulator columns: 3 per 768-col drain window
    n_acc = 3 * sum(-(-c * H // 768) for c in sizes)

    nc = bacc.Bacc("TRN2")
    pred = nc.declare_dram_parameter("predictions", [n_maps, H, H], F32, isOutput=False)
    targ = nc.declare_dram_parameter("targets", [n_maps, H, H], F32, isOutput=False)
    vband = nc.declare_dram_parameter("vband", [H, 2 * H], BF16, isOutput=False)
    outd = nc.declare_dram_parameter("out", [H, n_acc], F32, isOutput=True)

    with tile.TileContext(nc) as tc:
        with (
            tc.tile_pool(name="io", bufs=2) as iop,
            tc.tile_pool(name="wk", bufs=2) as wk,
            tc.tile_pool(name="acc", bufs=1) as accp,
            tc.tile_pool(name="psum", bufs=4, space="PSUM") as psp,
        ):
            acc = accp.tile([H, n_acc], F32, tag="acc", name="acc")
            # band stationaries (bf16, converted host-side)
            vb = accp.tile([H, 2 * H], BF16, tag="vb", name="vb")
            nc.sync.dma_start(out=vb[:], in_=vband[:])
            b_gs = []
            for k in range(2):
                bg = accp.tile([H, cm * H + 4], BF16, tag=f"b_g{k}", name=f"b_g{k}")[:]
                nc.gpsimd.memset(bg, 0.0)
                b_gs.append(bg)

            tiles = {}

            def phase1(ci, m0, c):
                F = c * H
                # sync + scalar HWDGE queues; a queue's sequencer blocks for
                # the whole transfer (~4us/786KB), and gpsimd SWDGE measured
                # strictly slower for these strided loads.
                tt = iop.tile([H, F], F32, tag="tt", name=f"tt{ci}")
                nc.sync.dma_start(
                    out=tt[:].rearrange("p (m w) -> p m w", w=H),
                    in_=targ[m0 : m0 + c].rearrange("m h w -> h m w"),
                )
                tp = iop.tile([H, F], F32, tag="tp", name=f"tp{ci}")
                nc.scalar.dma_start(
                    out=tp[:].rearrange("p (m w) -> p m w", w=H),
                    in_=pred[m0 : m0 + c].rearrange("m h w -> h m w"),
                )
                tt, tp = tt[:], tp[:]
                b_g = b_gs[ci % 2]
                amy = wk.tile([H, F], BF16, tag="amy", name=f"amy{ci}")[:]
                # amy = 2.1 - tt (ACT Copy with free affine; Pool is ~15x too
                # slow for full-res elementwise and wrecks DVE SBUF ports)
                nc.scalar.activation(amy, tt, AF.Copy, bias=2.1, scale=-1.0)
                # dm first in DVE order: b would chain behind amy (ACT) and
                # stall the in-order DVE queue at startup
                dm = wk.tile([H, F], BF16, tag="dm", name=f"dm{ci}")[:]
                nc.vector._custom_dve(
                    ops["AWL_ABSDM"], out=dm, in0=tp, in1=tt, s0=0.004, imm2=2.0
                )
                # b = [t >= 0.2] = [amy <= 1.9] into the padded b_g (bf16 4x)
                nc.vector.tensor_scalar(
                    b_g[:, 2 : F + 2], amy, 1.9, None, ALU.is_le
                )
                tiles[ci] = (dm, amy)

            def phase2(ci, m0, c):
                F = c * H
                dm, amy = tiles[ci]

                def T(tag):
                    return wk.tile([H, F], BF16, tag=tag, name=f"{tag}{ci}")[:]

                lnd, q, eq, sS, corr, l14 = (
                    T("lnd"), T("q"), T("eq"), T("sS"), T("corr"), T("l14"),
                )
                nc.scalar.activation(lnd, dm, AF.Ln, scale=0.5)
                nc.vector.tensor_tensor(q, amy, lnd, ALU.mult)
                nc.scalar.activation(eq, q, AF.Exp)
                nc.scalar.activation(sS, eq, AF.Ln, bias=1.0)
                nc.vector._custom_dve(
                    ops["AWL_CORR"], out=corr, in0=dm, in1=amy,
                    s0=CC0 + 2.1 * CC1, s1=-CC1,
                )
                nc.vector.tensor_tensor(l14, sS, corr, ALU.add)
                tiles[ci] = l14

            kctr = [0]

            def phase3(ci, m0, c):
                F = c * H
                l14 = tiles.pop(ci)
                b_g = b_gs[ci % 2]
                waste = wk.tile([H, F], BF16, tag="waste", name=f"waste{ci}")[:]
                # 768-col drain windows: 2-bank PSUM tiles, 4 in flight; fewer
                # drain dispatches than per-512 while keeping PE/DVE pipelined
                for w0 in range(0, F, 768):
                    ww = min(768, F - w0)
                    ps = psp.tile([H, ww], F32, tag="ps", name=f"ps{ci}_{w0}")
                    for s0 in range(0, ww, 512):
                        cw = min(512, ww - s0)
                        pw = ps[:, s0 : s0 + cw]
                        c0 = w0 + s0
                        nc.tensor.matmul(
                            pw, vb[:, 0:H], b_g[:, c0 + 1 : c0 + 1 + cw],
                            start=True, stop=False,
                        )
                        nc.tensor.matmul(
                            pw, vb[:, H : 2 * H], b_g[:, c0 + 2 : c0 + 2 + cw],
                            start=False, stop=False,
                        )
                        nc.tensor.matmul(
                            pw, vb[:, 0:H], b_g[:, c0 + 3 : c0 + 3 + cw],
                            start=False, stop=True,
                        )
                    k = kctr[0]
                    kctr[0] += 3
                    lsl = l14[:, w0 : w0 + ww]
                    wsl = waste[:, w0 : w0 + ww]
                    # main drain: sum ((ps>=0.5)+0.1)*l14 over the window
                    nc.vector._custom_dve(
                        ops["AWL_WACC"], out=wsl, in0=ps[:], in1=lsl,
                        s0=0.5, s1=0.1, accum_out=acc[:, k : k + 1],
                    )
                    # border cols keep b: undo the ps-thresholded term, add
                    # the b-thresholded one (cols {0,127} of each map via
                    # ::127).  These hide in pipeline gaps — dropping them
                    # saved nothing and cost 5x accuracy (measured).
                    ps3 = ps[:].rearrange("p (m w) -> p m w", w=H)[:, :, ::H - 1]
                    l3 = lsl.rearrange("p (m w) -> p m w", w=H)[:, :, ::H - 1]
                    w3 = wsl.rearrange("p (m w) -> p m w", w=H)[:, :, ::H - 1]
                    b3 = b_g[:, w0 + 2 : w0 + 2 + ww].rearrange(
                        "p (m w) -> p m w", w=H
                    )[:, :, ::H - 1]
                    nc.vector._custom_dve(
                        ops["AWL_WACCN"], out=w3, in0=ps3, in1=l3,
                        s0=0.5, s1=0.1, accum_out=acc[:, k + 1 : k + 2],
                    )
                    nc.vector._custom_dve(
                        ops["AWL_WACC"], out=w3, in0=b3, in1=l3,
                        s0=0.5, s1=0.1, accum_out=acc[:, k + 2 : k + 3],
                    )

            # 3-deep software pipeline
            ksplit = [0]
            for i in range(nch + 2):
                if i < nch:
                    phase1(i, *chunks[i])
                if 1 <= i <= nch:
                    phase2(i - 1, *chunks[i - 1])
                if i >= 2:
                    phase3(i - 2, *chunks[i - 2])
                if i == nch:
                    # store the finished first half of acc early so only a
                    # small strip remains after the last drain
                    ksplit[0] = kctr[0]
                    nc.sync.dma_start(
                        out=outd[:, : ksplit[0]], in_=acc[:, : ksplit[0]]
                    )
            nc.sync.dma_start(
                out=outd[:, ksplit[0] :], in_=acc[:, ksplit[0] :]
            )
    nc.compile()
    return nc


_TRACE = {"enabled": False, "last": None}


def kernel(predictions, targets):
    from concourse.bass_utils import run_bass_kernel_spmd

    preds = np.ascontiguousarray(predictions, dtype=np.float32)
    targs = np.ascontiguousarray(targets, dtype=np.float32)
    B = preds.shape[0]
    import ml_dtypes
    vband = make_vband().astype(ml_dtypes.bfloat16)
    in_maps = [
        {"predictions": preds[i], "targets": targs[i], "vband": vband}
        for i in range(N_CORES)
    ]
    nc = build_nc()
    kwargs = {}
    if _TRACE["enabled"]:
        kwargs = {"trace": True}
    try:
        res = run_bass_kernel_spmd(nc, in_maps, core_ids=list(range(N_CORES)), **kwargs)
    except Exception:
        if not kwargs:
            raise
        res = run_bass_kernel_spmd(nc, in_maps, core_ids=list(range(N_CORES)))
    _TRACE["last"] = res
    tot = 0.0
    for r in res.results:
        o = np.asarray(r["out"], dtype=np.float64)
        tot += 140.0 * o.sum()
    n_total = B * N_MAPS * H * H
    return np.float32(tot / n_total)


# revision 32
# speedup vs baseline: 1.1459x; 1.0665x over previous
"""AdaptiveWingLoss on 8 TRN2 NeuronCores (Bass/Tile) — "K1" tangent-correction
formulation.

Shards batch (8) across cores; each core computes the weighted loss sum over
its 68 maps of 128x128; host combines partial sums into the mean.

Math (ALPHA=2.1, OMEGA=14, THETA=0.5, EPS=1, W=10), amy = 2.1 - t:
  dY   = max(|p - t|, 0.004);  dm = 2*dY
  lnd  = ln(dY) = Ln(0.5*dm)
  q    = amy*lnd  (<= 0);  eq = exp(q) = dY**amy;  sS = ln(1+eq) = l14_small
  The large branch (dY >= 0.5) is the tangent line of sS at dY=0.5, so
  l14 = sS + corr with corr = relu(dm-1)^2 * (c0 + c1*t) — a least-squares fit
  of the tangency gap (|fit err| < 9e-3, final mean rel err ~2e-5).
  No branch select needed: relu(dm-1)^2 vanishes on the small branch.
  wfb  = 3x3 binary dilation of b=[t>=0.2] (borders keep b), w = wfb + 0.1
  loss = 140 * sum(w * l14) / N

Engines: ACT does {Copy(amy), Ln, Exp, Ln} — one table set, one
ACT_TABLE_LOAD, no phase gates. DVE does 3 full custom 1x passes (ABSDM, CORR,
WACC drain) plus stock bf16 ops (b threshold 4x; q, l14 at 2x). GpSimd (Pool)
does only the b_g pad memsets — its elementwise rate is ~15x too slow for
full-res work and its SBUF traffic degrades concurrent DVE perf modes. PE does
the 3x3 OR-dilation as 3 shifted band-matrix matmuls per 512-col window
(center matrix carries the border-row identity). The WACC drain fuses
threshold+weight+multiply+reduce: ((ps>=0.5)+0.1)*l14 with accum=ADD over
768-col PSUM windows, so the weighted sum falls out of the drain directly;
border columns (which must keep b instead of the dilated value) are patched by
two tiny ::127-strided passes per window (subtract the wrong term, add the
right one). DMA: targets on sync HWDGE, predictions on scalar HWDGE — both
queues' sequencers block for the whole transfer, so loads never share a queue
with the hot compute engines (DVE), and gpsimd SWDGE measured strictly slower
for these [h, m, w]-strided loads.

Measured (core 0): ~70.5us exec; DVE busy ~46us (the critical engine), ACT
~36.5us, PE ~26us, DMA ~30us/engine. Baseline before rework: 90.9us.
"""

import numpy as np

import concourse.bass as bass
import concourse.tile as tile
from concourse import bacc
from concourse import mybir

F32 = mybir.dt.float32
BF16 = mybir.dt.bfloat16
AF = mybir.ActivationFunctionType
ALU = mybir.AluOpType

H = 128
N_CORES = 8
N_MAPS = 68  # per core (68 landmarks x 1 batch element)
# (2,12,...): best measured schedule.  Graded ramps (6,8,12,... / 2,4,8,...)
# trade startup vs early-gap time and measured equal or worse; more chunks
# add per-op dispatch overhead.
SIZES = (2, 12, 12, 12, 12, 12, 6)

# corr = relu(dm-1)^2 * (CC0 + CC1*t), least-squares fit of the tangency gap
# over the (t, dY) distribution. Reparametrized in amy = 2.1 - t at the call
# site: s0 = CC0 + 2.1*CC1, s1 = -CC1.
CC0 = -0.097028
CC1 = 0.140812

_ALLOWED_TABLES = ("natural_log_exp_and_others",)
_patched_tables = False
_custom_ops = {}


def _register_custom_ops():
    """Four fused DVE ops:
    AWL_ABSDM:  out = max(max(in0-in1, in1-in0), s0) * imm2     (dm = 2*dY)
    AWL_CORR:   out = relu(in0-1)^2 * (s1*in1 + s0)             (tangency fix)
    AWL_WACC:   out = ((in0>=s0)+s1)*in1, accum_out = sum(out)  (weighted drain)
    AWL_WACCN:  out = -((in0>=s0)+s1)*in1, accum_out = sum(out) (border undo)
    """
    if _custom_ops:
        return _custom_ops
    from concourse import dve_ops
    from concourse.dve_spec import (
        Spec, Src0, Src1, C0, C1, C2, Zero, One, maxx, relu, lower, AluOp,
    )
    from concourse.dve_uop import DveOpSpec

    r = relu(Src0 - One)
    defs = [
        (
            "AWL_ABSDM",
            Spec(
                body=maxx(maxx(Src0 - Src1, Src1 - Src0), C0) * C2,
                reference=lambda in0, in1, s0, s1, imm2: (
                    np.maximum(np.abs(in0.astype(np.float32) - in1), s0) * imm2
                ).astype(np.float32),
            ),
        ),
        (
            "AWL_CORR",
            Spec(
                body=(r * (C1 * Src1 + C0)) * r,
                reference=lambda in0, in1, s0, s1, imm2: (
                    (np.maximum(in0.astype(np.float32) - 1.0, 0.0) ** 2)
                    * (s1 * in1 + s0)
                ).astype(np.float32),
            ),
        ),
        (
            "AWL_WACC",
            Spec(
                body=((Src0 >= C0) + C1) * Src1,
                accum=AluOp.ADD,
                reference=lambda in0, in1, s0, s1, imm2: (
                    ((in0 >= s0).astype(np.float32) + s1) * in1
                ).astype(np.float32),
            ),
        ),
        (
            "AWL_WACCN",
            Spec(
                body=(Zero - ((Src0 >= C0) + C1)) * Src1,
                accum=AluOp.ADD,
                reference=lambda in0, in1, s0, s1, imm2: (
                    -((in0 >= s0).astype(np.float32) + s1) * in1
                ).astype(np.float32),
            ),
        ),
    ]
    for name, spec in defs:
        if name in dve_ops._SUB_OPCODE_FOR_NAME:
            _custom_ops[name] = next(o for o in dve_ops.OPS if o.name == name)
            continue
        opcode = dve_ops._CUSTOM_DVE_ROW_BASE + len(dve_ops.OPS)
        assert opcode < 0x20
        shas = {}
        for ver in ("v3", "v4"):
            ds = DveOpSpec(
                name=name, opcode=opcode, uops=lower(spec, ver=ver), rd1_en=True
            )
            shas[ver] = ds.sha(ver)
        dve_ops._SUB_OPCODE_FOR_NAME[name] = opcode
        op = dve_ops.DveOp(name, spec, subdim=False, uops_sha=shas)
        dve_ops.OPS.append(op)
        dve_ops.CUSTOM_DVE_SPECS[name] = spec
        _custom_ops[name] = op
    return _custom_ops


def _patch_act_tables():
    """Restrict bacc's activation-set choices to the single set we use
    (Ln+Exp) so the fixpoint pass emits exactly one ACT_TABLE_LOAD."""
    global _patched_tables
    if _patched_tables:
        return
    orig = bacc.get_activation_tables

    def patched(arch):
        tabs = orig(arch)
        return {k: (v if k in _ALLOWED_TABLES else set()) for k, v in tabs.items()}

    bacc.get_activation_tables = patched
    _patched_tables = True


def make_vband():
    """[128, 256] stationaries for the vertical OR:
    cols 0:128   M_side   = 3-row band, interior output rows 1..126 only
    cols 128:256 M_center = band + identity at rows 0/127 (border rows keep b)."""
    v = np.zeros((H, 2 * H), dtype=np.float32)
    for i in range(1, H - 1):
        for k in (i - 1, i, i + 1):
            v[k, i] = 1.0
            v[k, H + i] = 1.0
    v[0, H + 0] = 1.0
    v[H - 1, H + H - 1] = 1.0
    return v


def build_nc(n_maps=N_MAPS, sizes=SIZES):
    _patch_act_tables()
    ops = _register_custom_ops()
    assert sum(sizes) == n_maps
    chunks = []
    m0 = 0
    for c in sizes:
        chunks.append((m0, c))
        m0 += c
    nch = len(chunks)
    cm = max(sizes)

    # accumulator columns: 3 per 768-col drain window
    n_acc = 3 * sum(-(-c * H // 768) for c in sizes)

    nc = bacc.Bacc("TRN2")
    pred = nc.declare_dram_parameter("predictions", [n_maps, H, H], F32, isOutput=False)
    targ = nc.declare_dram_parameter("targets", [n_maps, H, H], F32, isOutput=False)
    vband = nc.declare_dram_parameter("vband", [H, 2 * H], BF16, isOutput=False)
    outd = nc.declare_dram_parameter("out", [H, n_acc], F32, isOutput=True)

    with tile.TileContext(nc) as tc:
        with (
            tc.tile_pool(name="io", bufs=2) as iop,
            tc.tile_pool(name="wk", bufs=3) as wk,
            tc.tile_pool(name="acc", bufs=1) as accp,
            tc.tile_pool(name="psum", bufs=4, space="PSUM") as psp,
        ):
            acc = accp.tile([H, n_acc], F32, tag="acc", name="acc")
            # band stationaries (bf16, converted host-side)
            vb = accp.tile([H, 2 * H], BF16, tag="vb", name="vb")
            nc.sync.dma_start(out=vb[:], in_=vband[:])
            b_gs = []
            for k in range(2):
                bg = accp.tile([H, cm * H + 4], BF16, tag=f"b_g{k}", name=f"b_g{k}")[:]
                nc.gpsimd.memset(bg, 0.0)
                b_gs.append(bg)

            tiles = {}

            def phase1(ci, m0, c):
                F = c * H
                # sync + scalar HWDGE queues; a queue's sequencer blocks for
                # the whole transfer (~4us/786KB), and gpsimd SWDGE measured
                # strictly slower for these strided loads.
                tt = iop.tile([H, F], F32, tag="tt", name=f"tt{ci}")
                nc.sync.dma_start(
                    out=tt[:].rearrange("p (m w) -> p m w", w=H),
                    in_=targ[m0 : m0 + c].rearrange("m h w -> h m w"),
                )
                tp = iop.tile([H, F], F32, tag="tp", name=f"tp{ci}")
                nc.scalar.dma_start(
                    out=tp[:].rearrange("p (m w) -> p m w", w=H),
                    in_=pred[m0 : m0 + c].rearrange("m h w -> h m w"),
                )
                tt, tp = tt[:], tp[:]
                b_g = b_gs[ci % 2]
                amy = wk.tile([H, F], BF16, tag="amy", name=f"amy{ci}")[:]
                # amy = 2.1 - tt (ACT Copy with free affine; Pool is ~15x too
                # slow for full-res elementwise and wrecks DVE SBUF ports)
                nc.scalar.activation(amy, tt, AF.Copy, bias=2.1, scale=-1.0)
                # dm first in DVE order: b would chain behind amy (ACT) and
                # stall the in-order DVE queue at startup
                dm = wk.tile([H, F], BF16, tag="dm", name=f"dm{ci}")[:]
                nc.vector._custom_dve(
                    ops["AWL_ABSDM"], out=dm, in0=tp, in1=tt, s0=0.004, imm2=2.0
                )
                # b = [t >= 0.2] = [amy <= 1.9] into the padded b_g (bf16 4x)
                nc.vector.tensor_scalar(
                    b_g[:, 2 : F + 2], amy, 1.9, None, ALU.is_le
                )
                tiles[ci] = (dm, amy)

            def phase2(ci, m0, c):
                F = c * H
                dm, amy = tiles[ci]

                def T(tag):
                    return wk.tile([H, F], BF16, tag=tag, name=f"{tag}{ci}")[:]

                lnd, q, eq, sS, corr, l14 = (
                    T("lnd"), T("q"), T("eq"), T("sS"), T("corr"), T("l14"),
                )
                nc.scalar.activation(lnd, dm, AF.Ln, scale=0.5)
                nc.vector.tensor_tensor(q, amy, lnd, ALU.mult)
                nc.scalar.activation(eq, q, AF.Exp)
                nc.scalar.activation(sS, eq, AF.Ln, bias=1.0)
                nc.vector._custom_dve(
                    ops["AWL_CORR"], out=corr, in0=dm, in1=amy,
                    s0=CC0 + 2.1 * CC1, s1=-CC1,
                )
                nc.vector.tensor_tensor(l14, sS, corr, ALU.add)
                tiles[ci] = l14

            kctr = [0]

            def phase3(ci, m0, c):
                F = c * H
                l14 = tiles.pop(ci)
                b_g = b_gs[ci % 2]
                waste = wk.tile([H, F], BF16, tag="waste", name=f"waste{ci}")[:]
                # 768-col drain windows: 2-bank PSUM tiles, 4 in flight; fewer
                # drain dispatches than per-512 while keeping PE/DVE pipelined
                for w0 in range(0, F, 768):
                    ww = min(768, F - w0)
                    ps = psp.tile([H, ww], F32, tag="ps", name=f"ps{ci}_{w0}")
                    for s0 in range(0, ww, 512):
                        cw = min(512, ww - s0)
                        pw = ps[:, s0 : s0 + cw]
                        c0 = w0 + s0
                        nc.tensor.matmul(
                            pw, vb[:, 0:H], b_g[:, c0 + 1 : c0 + 1 + cw],
                            start=True, stop=False,
                        )
                        nc.tensor.matmul(
                            pw, vb[:, H : 2 * H], b_g[:, c0 + 2 : c0 + 2 + cw],
                            start=False, stop=False,
                        )
                        nc.tensor.matmul(
                            pw, vb[:, 0:H], b_g[:, c0 + 3 : c0 + 3 + cw],
                            start=False, stop=True,
                        )
                    k = kctr[0]
                    kctr[0] += 3
                    lsl = l14[:, w0 : w0 + ww]
                    wsl = waste[:, w0 : w0 + ww]
                    # main drain: sum ((ps>=0.5)+0.1)*l14 over the window
                    nc.vector._custom_dve(
                        ops["AWL_WACC"], out=wsl, in0=ps[:], in1=lsl,
                        s0=0.5, s1=0.1, accum_out=acc[:, k : k + 1],
                    )
                    # border cols keep b: undo the ps-thresholded term, add
                    # the b-thresholded one (cols {0,127} of each map via
                    # ::127).  These hide in pipeline gaps — dropping them
                    # saved nothing and cost 5x accuracy (measured).
                    ps3 = ps[:].rearrange("p (m w) -> p m w", w=H)[:, :, ::H - 1]
                    l3 = lsl.rearrange("p (m w) -> p m w", w=H)[:, :, ::H - 1]
                    w3 = wsl.rearrange("p (m w) -> p m w", w=H)[:, :, ::H - 1]
                    b3 = b_g[:, w0 + 2 : w0 + 2 + ww].rearrange(
                        "p (m w) -> p m w", w=H
                    )[:, :, ::H - 1]
                    nc.vector._custom_dve(
                        ops["AWL_WACCN"], out=w3, in0=ps3, in1=l3,
                        s0=0.5, s1=0.1, accum_out=acc[:, k + 1 : k + 2],
                    )
                    nc.vector._custom_dve(
                        ops["AWL_WACC"], out=w3, in0=b3, in1=l3,
                        s0=0.5, s1=0.1, accum_out=acc[:, k + 2 : k + 3],
                    )

            # 3-deep software pipeline
            ksplit = [0]
            for i in range(nch + 2):
                if i < nch:
                    phase1(i, *chunks[i])
                if 1 <= i <= nch:
                    phase2(i - 1, *chunks[i - 1])
                if i >= 2:
                    phase3(i - 2, *chunks[i - 2])
                if i == nch:
                    # store the finished first half of acc early so only a
                    # small strip remains after the last drain
                    ksplit[0] = kctr[0]
                    nc.sync.dma_start(
                        out=outd[:, : ksplit[0]], in_=acc[:, : ksplit[0]]
                    )
            nc.sync.dma_start(
                out=outd[:, ksplit[0] :], in_=acc[:, ksplit[0] :]
            )
    nc.compile()
    return nc


_TRACE = {"enabled": False, "last": None}


def kernel(predictions, targets):
    from concourse.bass_utils import run_bass_kernel_spmd

    preds = np.ascontiguousarray(predictions, dtype=np.float32)
    targs = np.ascontiguousarray(targets, dtype=np.float32)
    B = preds.shape[0]
    import ml_dtypes
    vband = make_vband().astype(ml_dtypes.bfloat16)
    in_maps = [
        {"predictions": preds[i], "targets": targs[i], "vband": vband}
        for i in range(N_CORES)
    ]
    nc = build_nc()
    kwargs = {}
    if _TRACE["enabled"]:
        kwargs = {"trace": True}
    try:
        res = run_bass_kernel_spmd(nc, in_maps, core_ids=list(range(N_CORES)), **kwargs)
    except Exception:
        if not kwargs:
            raise
        res = run_bass_kernel_spmd(nc, in_maps, core_ids=list(range(N_CORES)))
    _TRACE["last"] = res
    tot = 0.0
    for r in res.results:
        o = np.asarray(r["out"], dtype=np.float64)
        tot += 140.0 * o.sum()
    n_total = B * N_MAPS * H * H
    return np.float32(tot / n_total)
